# revision 27
# baseline (speedup 1.0000x reference)
"""Trainium2 Bass kernel for sparse causal attention (nn_CausalAttentionKV).

Reference computation (fp32, single device):
    q_all = x @ Wq + bq ; k_all = x @ Wk + bk ; v_all = x @ Wv + bv
    q = gather(q_all, query_idx)        # (B, M, D) selected query rows
    att = softmax(mask(q k^T / sqrt(hd)))   # per-query causal mask t <= qidx[m]
    y = (att v) @ Wo + bo

Shapes: B=4, T=4096, D=2048, n_head=16, hd=128, M=512.

Sharding (8 cores): core = 2*b + g  handles batch b and head-group g
(8 heads = 1024 feature cols).  Q/K/V projections are column-parallel,
out-proj is row-parallel; the two partial outputs per batch are summed
on the host.  All matmul inputs are bf16 (fp32 PSUM accumulation).

Schedule (single fused sweep, flash-attention style): Q projection
first (d-outer so the first matmul only needs ~384KB of input), then
ONE pass over x in 512-column t-chunks.  Each t-chunk projects K and V
for all 8 heads, immediately computes the score chunks against the
resident Q^T, exps them (causal masking is applied as a 0/1 multiply
on the bf16 exp output, off the scalar engine's critical path), and
runs the P@V / row-sum matmuls lagged by ~4 head-groups so the tensor
engine never waits on the scalar-engine exp.  K/V are consumed
in-chunk: nothing is spilled to DRAM and the attention's scalar(exp)
and vector(mask/esum/accumulate) work - which made a separate
attention phase scalar-bound - hides under the 55us/chunk projection
matmul stream.  P@V partial sums and softmax denominators accumulate
in SBUF fp32 (vector adds) since PSUM (8 banks) is fully committed to
projection/score/PV pipelining.  Normalization is deferred to the end
(one bf16 broadcast matmul per head).  Input loading is split across
the sync/scalar/gpsimd DMA queues to cut the startup serialization;
the y output alternates sync/scalar queues to shrink the drain tail.
Per-chunk score skip bounds (flo/fhi) avoid fully-masked score work
(~47% of attention) exactly as in the reference semantics.
"""

import sys
import types
from contextlib import ExitStack

import numpy as np
import ml_dtypes

import concourse.bass as bass
import concourse.tile as tile
import concourse.mybir as mybir
from concourse import bacc
from concourse.bass_utils import run_bass_kernel_spmd

BF16 = mybir.dt.bfloat16
F32 = mybir.dt.float32
NPBF = ml_dtypes.bfloat16

B, T, D = 4, 4096, 2048
NH, HD, M = 16, 128, 512
NHG = 8            # heads per core (group)
DG = NHG * HD      # 1024 feature cols per core
NT = T // 128      # 32 t-chunks
ND = D // 128      # 16 d-chunks
KTS = 512          # t columns per sweep step
NTS = T // KTS     # 8 sweep steps


def _install_ntff_hook():
    """Register the axon NTFF profiling hook if the image's antenv lacks it."""
    try:
        from antenv.axon_hooks import get_axon_ntff_profile_hook  # noqa: F401
        return
    except ImportError:
        pass
    try:
        import antenv
        from trn_agent_boot.trn_boot import _ntff_profile_via_ctypes

        mod = types.ModuleType("antenv.axon_hooks")
        hook = [None]
        mod.set_axon_ntff_profile_hook = lambda h: hook.__setitem__(0, h)
        mod.get_axon_ntff_profile_hook = lambda: hook[0]
        sys.modules["antenv.axon_hooks"] = mod
        antenv.axon_hooks = mod
        mod.set_axon_ntff_profile_hook(
            _ntff_profile_via_ctypes("/opt/axon/libaxon_pjrt.so")
        )
    except Exception:
        pass


def build_program(flo, fhi):
    """Build the per-core Bass program.

    flo[i]: first m column with any allowed key in t-chunk i (cols below
            are fully masked there -> never computed).
    fhi[i]: first m column fully allowed in t-chunk i (cols beyond need
            no masking).
    Both are unions over the 4 batches so one program serves all cores.
    """
    nc = bacc.Bacc("TRN2", target_bir_lowering=False, debug=False)

    xT = nc.dram_tensor("xT", [D, T], BF16, kind="ExternalInput")
    xqT = nc.dram_tensor("xqT", [D, M], BF16, kind="ExternalInput")
    wk = nc.dram_tensor("wk", [D, DG], BF16, kind="ExternalInput")
    wv = nc.dram_tensor("wv", [D, DG], BF16, kind="ExternalInput")
    wq = nc.dram_tensor("wq", [D, DG], BF16, kind="ExternalInput")
    wo = nc.dram_tensor("wo", [DG, D], BF16, kind="ExternalInput")
    mwid = [max(fhi[i] - flo[i], 0) for i in range(NT)]
    moff = [0] * NT
    for i in range(1, NT):
        moff[i] = moff[i - 1] + mwid[i - 1]
    SW = max(moff[-1] + mwid[-1], 1)
    maskd = nc.dram_tensor("mask", [128, SW], BF16, kind="ExternalInput")
    bks = nc.dram_tensor("bks", [128, NHG], F32, kind="ExternalInput")
    bqs = nc.dram_tensor("bqs", [128, NHG], F32, kind="ExternalInput")
    y = nc.dram_tensor("y", [M, D], F32, kind="ExternalOutput")

    # (c*128+p, t) views for chunked DMA
    xTr = xT.rearrange("(c p) t -> p c t", p=128)
    xqTr = xqT.rearrange("(c p) t -> p c t", p=128)
    wkr = wk.rearrange("(c p) t -> p c t", p=128)
    wvr = wv.rearrange("(c p) t -> p c t", p=128)
    wqr = wq.rearrange("(c p) t -> p c t", p=128)
    wor = wo.rearrange("(c p) t -> p c t", p=128)

    active = [[i for i in range(4 * ts, 4 * ts + 4) if flo[i] < M]
              for ts in range(NTS)]

    with ExitStack() as ctx:
        tc = ctx.enter_context(tile.TileContext(nc))

        # ---- persistent tiles --------------------------------------
        persist = ctx.enter_context(tc.tile_pool(name="persist", bufs=1))
        qt_t = [persist.tile([128, M], BF16, name=f"qt{j}", tag=f"qt{j}") for j in range(NHG)]
        po_acc = [persist.tile([128, M], F32, name=f"po{j}", tag=f"po{j}") for j in range(NHG)]
        l_acc = [persist.tile([1, M], F32, name=f"l{j}", tag=f"l{j}") for j in range(NHG)]
        bias_k = persist.tile([128, NHG], F32, name="bias_k", tag="bias_k")
        bias_q = persist.tile([128, NHG], F32, name="bias_q", tag="bias_q")
        zbias = persist.tile([128, 1], F32, name="zbias", tag="zbias")
        ones_c = persist.tile([128, 1], BF16, name="ones_c", tag="ones_c")
        ones_r = persist.tile([1, 128], BF16, name="ones_r", tag="ones_r")
        # mask multiplicands (0/1): all chunk windows packed in one tile
        mask_t = persist.tile([128, SW], BF16, name="maskp", tag="maskp")

        nc.sync.dma_start(bias_k[:], bks[:])
        nc.sync.dma_start(bias_q[:], bqs[:])
        nc.vector.memset(zbias[:], 0.0)
        nc.vector.memset(ones_c[:], 1.0)
        nc.vector.memset(ones_r[:], 1.0)

        # K/V weights + first x chunk prefetch on the scalar HW queue;
        # masks on the gpsimd SW queue (small, needed ~55us in).
        wkp = ctx.enter_context(tc.tile_pool(name="wkp", bufs=1))
        wk_t = [wkp.tile([128, 4, DG], BF16, name=f"wk{d}", tag=f"wk{d}") for d in range(4)]
        wv_t = [wkp.tile([128, 4, DG], BF16, name=f"wv{d}", tag=f"wv{d}") for d in range(4)]
        xtp = ctx.enter_context(tc.tile_pool(name="xtp", bufs=2))
        xt0 = [xtp.tile([128, 4, KTS], BF16, name=f"xt{d}", tag=f"xt{d}") for d in range(4)]

        # ---- phase A-Q: Qt[j] = ((xq @ wq_j + bq_j)/sqrt(hd))^T ----
        # d-outer: step d needs only xq_d (128KB) + wq_d (256KB), so the
        # first matmul launches ~3us in; all 8 head PSUM groups stay open.
        inv_s = 1.0 / float(np.sqrt(HD))
        with (
            nc.named_scope("phase_AQ"),
            tc.tile_pool(name="wqp", bufs=1) as wqp,
            tc.tile_pool(name="pq", bufs=1, space="PSUM") as pqp,
        ):
            xq_t, wq_t = [], []
            for d in range(ND):
                xq_t.append(wqp.tile([128, M], BF16, name=f"xq{d}", tag=f"xq{d}"))
                wq_t.append(wqp.tile([128, DG], BF16, name=f"wq{d}", tag=f"wq{d}"))
                nc.sync.dma_start(xq_t[d][:], xqTr[:, d, :])
                (nc.sync if d % 2 == 0 else nc.scalar).dma_start(wq_t[d][:], wqr[:, d, :])
                if d in (5, 8, 11, 14):
                    nc.sync.dma_start(xt0[(d - 5) // 3][:],
                                      xTr[:, (d - 5) // 3 * 4: (d - 5) // 3 * 4 + 4, 0:KTS])
                if d in (4, 7, 10, 13):
                    dd = (d - 4) // 3
                    nc.scalar.dma_start(wk_t[dd][:], wkr[:, 4 * dd: 4 * dd + 4, :])
            nc.sync.dma_start(mask_t[:], maskd[:])
            for d in range(2):
                nc.scalar.dma_start(wv_t[d][:], wvr[:, 4 * d: 4 * d + 4, :])
            for d in range(2, 4):
                nc.sync.dma_start(wv_t[d][:], wvr[:, 4 * d: 4 * d + 4, :])
            pq = [pqp.tile([128, M], F32, name=f"pq{j}", tag=f"pq{j}") for j in range(NHG)]
            for d in range(ND):
                for j in range(NHG):
                    nc.tensor.matmul(
                        pq[j][:],
                        wq_t[d][:, j * 128: (j + 1) * 128],
                        xq_t[d][:],
                        start=(d == 0),
                        stop=(d == ND - 1),
                        skip_group_check=True,
                    )
            for j in range(NHG):
                nc.scalar.activation(
                    qt_t[j][:], pq[j][:],
                    mybir.ActivationFunctionType.Identity,
                    scale=inv_s, bias=bias_q[:, j: j + 1],
                )

        # ---- fused sweep: K/V projection + attention per t-chunk ----
        wop = ctx.enter_context(tc.tile_pool(name="wop", bufs=1))
        wo_t = [wop.tile([128, 2, D], BF16, name="wo0", tag="wo0")]

        with (
            nc.named_scope("phase_sweep"),
            tc.tile_pool(name="kst", bufs=1) as kstp,
            tc.tile_pool(name="vst", bufs=1) as vstp,
            tc.tile_pool(name="esb", bufs=3) as esbp,
            tc.tile_pool(name="essum", bufs=1) as esump,
            tc.tile_pool(name="lsb", bufs=1) as lsbp,
            tc.tile_pool(name="pkv", bufs=2, space="PSUM") as pkvp,
            tc.tile_pool(name="ps", bufs=4, space="PSUM") as psp,
            tc.tile_pool(name="ppv", bufs=1, space="PSUM") as ppvp,
            tc.tile_pool(name="prs", bufs=1, space="PSUM") as prsp,
        ):
            pend = []      # deferred PV/rowsum jobs
            v_live = {}    # ts -> list of v tiles

            def emit_norm(j):
                # ot[j] = po[j] / l[j], reusing the dead qt tile as ot
                linv32 = lsbp.tile([1, M], F32, name="linv32", tag=f"linv32{j % 2}")
                nc.vector.reciprocal_approx_fast(linv32[:], l_acc[j][:])
                linv = lsbp.tile([1, M], BF16, name="linv", tag=f"linv{j % 2}")
                nc.vector.tensor_copy(linv[:], linv32[:])
                pb = psp.tile([128, M], F32, name="pb", tag="s")
                nc.tensor.matmul(pb[:], ones_r[:], linv[:], start=True,
                                 stop=True, skip_group_check=True)
                nc.vector.tensor_mul(qt_t[j][:], po_acc[j][:], pb[:])

            def flush_pv(norm_inline=True):
                j, ts0, items = pend.pop(0)
                lo = items[0][1]
                vts = v_live[ts0]
                pvt = ppvp.tile([128, M], F32, name="pv", tag="pv")
                for k, (i, lo_i, et) in enumerate(items):
                    nc.tensor.matmul(
                        pvt[:, lo_i:M],
                        vts[i - 4 * ts0][:, j * 128: (j + 1) * 128],
                        et[:, lo_i:M],
                        start=(k == 0),
                        stop=(k == len(items) - 1),
                        skip_group_check=True,
                    )
                # esum: right-aligned sum of the chunk exps (bf16, DVE 2x)
                est = esump.tile([128, M], BF16, name="esum", tag="esum")
                nc.vector.tensor_copy(est[:, lo:M], items[0][2][:, lo:M])
                for (i, lo_i, et) in items[1:]:
                    nc.vector.tensor_add(est[:, lo_i:M], est[:, lo_i:M], et[:, lo_i:M])
                rst = prsp.tile([1, M], F32, name="rs", tag="rs")
                nc.tensor.matmul(rst[:, lo:M], ones_c[:], est[:, lo:M],
                                 start=True, stop=True, skip_group_check=True)
                if ts0 == 0:
                    nc.vector.tensor_copy(po_acc[j][:, lo:M], pvt[:, lo:M])
                    nc.vector.tensor_copy(l_acc[j][:, lo:M], rst[:, lo:M])
                else:
                    nc.vector.tensor_add(po_acc[j][:, lo:M], po_acc[j][:, lo:M], pvt[:, lo:M])
                    nc.vector.tensor_add(l_acc[j][:, lo:M], l_acc[j][:, lo:M], rst[:, lo:M])
                if ts0 == NTS - 1 and norm_inline:
                    emit_norm(j)
                return j, ts0

            for ts in range(NTS):
                if ts == 0:
                    xt_t = xt0
                else:
                    xt_t = [xtp.tile([128, 4, KTS], BF16, name=f"xt{d}", tag=f"xt{d}") for d in range(4)]
                    for d in range(4):
                        nc.sync.dma_start(
                            xt_t[d][:], xTr[:, 4 * d: 4 * d + 4, ts * KTS: (ts + 1) * KTS]
                        )
                if ts == NTS - 1:
                    # first wo slice rides under the last sweep step
                    nc.scalar.dma_start(wo_t[0][:], wor[:, 0:2, :])

                # K^T for all heads: kj[j] = (wk_j^T x)[hd, t]
                kj = []
                for j in range(NHG):
                    pk = pkvp.tile([128, KTS], F32, name="pk", tag="pkv")
                    for d in range(ND):
                        nc.tensor.matmul(
                            pk[:],
                            wk_t[d // 4][:, d % 4, j * 128: (j + 1) * 128],
                            xt_t[d // 4][:, d % 4, :],
                            start=(d == 0),
                            stop=(d == ND - 1),
                        )
                    kt = kstp.tile([128, KTS], BF16, name=f"k{j}", tag=f"k{j}")
                    nc.scalar.activation(
                        kt[:], pk[:],
                        mybir.ActivationFunctionType.Identity,
                        bias=bias_k[:, j: j + 1],
                    )
                    kj.append(kt)
                    if pend:
                        flush_pv()

                # V: (t, DG) tiles for this step
                vts = []
                for u in range(KTS // 128):
                    vt = vstp.tile([128, DG], BF16, name=f"v{u}", tag=f"v{u}")
                    for f in range(2):
                        pv = pkvp.tile([128, 512], F32, name="pvp", tag="pkv")
                        for d in range(ND):
                            nc.tensor.matmul(
                                pv[:],
                                xt_t[d // 4][:, d % 4, u * 128: (u + 1) * 128],
                                wv_t[d // 4][:, d % 4, f * 512: (f + 1) * 512],
                                start=(d == 0),
                                stop=(d == ND - 1),
                            )
                        nc.vector.tensor_copy(vt[:, f * 512: (f + 1) * 512], pv[:])
                    vts.append(vt)
                v_live[ts] = vts
                v_live.pop(ts - 2, None)

                # scores + exp per head; PV lagged 4 head-groups
                for j in range(NHG):
                    items = []
                    for c, i in enumerate(active[ts]):
                        lo_i = flo[i]
                        st = psp.tile([128, M], F32, name="s", tag="s")
                        nc.tensor.matmul(
                            st[:, lo_i:M],
                            kj[j][:, (i % 4) * 128: (i % 4 + 1) * 128],
                            qt_t[j][:, lo_i:M],
                            start=True, stop=True, skip_group_check=True,
                        )
                        et = esbp.tile([128, M], BF16, name="e", tag=f"e{c}")
                        nc.scalar.activation(
                            et[:, lo_i:M], st[:, lo_i:M],
                            mybir.ActivationFunctionType.Exp,
                            bias=zbias[:],
                        )
                        if fhi[i] > lo_i:
                            nc.vector.tensor_mul(
                                et[:, lo_i: fhi[i]],
                                et[:, lo_i: fhi[i]],
                                mask_t[:, moff[i]: moff[i] + mwid[i]],
                            )
                        items.append((i, lo_i, et))
                    if items:
                        pend.append((j, ts, items))
                    if j >= 2 and pend:
                        flush_pv()

            # drain: flushes first so the recip chains pipeline, then norms
            tail_norms = []
            while pend:
                j, ts0 = flush_pv(norm_inline=False)
                if ts0 == NTS - 1:
                    tail_norms.append(j)
            for j in tail_norms:
                emit_norm(j)

        # remaining out-proj weights (space freed by the sweep pools)
        for dd in range(1, 4):
            wo_t.append(wop.tile([128, 2, D], BF16, name=f"wo{dd}", tag=f"wo{dd}"))
            nc.scalar.dma_start(wo_t[dd][:], wor[:, 2 * dd: 2 * dd + 2, :])


        # ---- phase C: y = O @ wo  (row-parallel partial) -----------
        with (
            nc.named_scope("phase_C"),
            tc.tile_pool(name="py", bufs=2, space="PSUM") as pyp,
            tc.tile_pool(name="ysb", bufs=3) as ysb,
        ):
            for mb in range(M // 128):
                for fp in range(D // 1024):
                    py = [
                        pyp.tile([128, 512], F32, name="py", tag=f"py{h}")
                        for h in range(2)
                    ]
                    for j in range(NHG):
                        for h in range(2):
                            fo = 2 * fp + h
                            nc.tensor.matmul(
                                py[h][:],
                                qt_t[j][:, mb * 128: (mb + 1) * 128],
                                wo_t[j // 2][:, j % 2, fo * 512: (fo + 1) * 512],
                                start=(j == 0),
                                stop=(j == NHG - 1),
                                skip_group_check=True,
                            )
                    for h in range(2):
                        ys = ysb.tile([128, 512], F32, name="ys", tag="ys")
                        nc.scalar.copy(ys[:], py[h][:])
                        eng = nc.sync if (2 * mb + fp + h) % 2 == 0 else nc.scalar
                        eng.dma_start(
                            y[
                                mb * 128: (mb + 1) * 128,
                                (2 * fp + h) * 512: (2 * fp + h + 1) * 512,
                            ],
                            ys[:],
                        )

    nc.compile()
    return nc


_cache = {}


def _get_program(flo, fhi):
    key = (tuple(flo), tuple(fhi))
    if key not in _cache:
        _cache[key] = build_program(list(flo), list(fhi))
    return _cache[key]


def _prep(inputs):
    x = np.asarray(inputs["x"], dtype=np.float32)
    qidx = np.asarray(inputs["query_idx"]).astype(np.int64)
    Wq = np.asarray(inputs["Wq"], dtype=np.float32)
    Wk = np.asarray(inputs["Wk"], dtype=np.float32)
    Wv = np.asarray(inputs["Wv"], dtype=np.float32)
    Wo = np.asarray(inputs["Wo"], dtype=np.float32)
    bq = np.asarray(inputs["bq"], dtype=np.float32)
    bk = np.asarray(inputs["bk"], dtype=np.float32)
    bv = np.asarray(inputs["bv"], dtype=np.float32)
    bo = np.asarray(inputs["bo"], dtype=np.float32)

    # Per-t-chunk skip bounds, union over batches.  flo[i] = first m that
    # attends into chunk i (everything below is fully masked there);
    # fhi[i] = one past the last m only partially covered by chunk i.
    # Computed positionally so they are correct even for unsorted
    # query_idx (just less effective at skipping).
    flo = [M] * NT
    fhi = [0] * NT
    for b in range(B):
        for i in range(NT):
            allowed = qidx[b] >= 128 * i          # chunk i not fully masked
            partial = qidx[b] < 128 * (i + 1)     # chunk i not fully allowed
            lo_b = int(np.argmax(allowed)) if allowed.any() else M
            hi_b = M - int(np.argmax(partial[::-1])) if partial.any() else 0
            flo[i] = min(flo[i], lo_b)
            fhi[i] = max(fhi[i], hi_b)

    # pack per-chunk mask windows [128, fhi-flo) into one [128, SW] tensor
    mwid = [max(fhi[i] - flo[i], 0) for i in range(NT)]
    moff = [0] * NT
    for i in range(1, NT):
        moff[i] = moff[i - 1] + mwid[i - 1]
    SW = max(moff[-1] + mwid[-1], 1)

    in_maps = []
    tgrid = np.arange(T)[:, None]
    for core in range(8):
        b, g = divmod(core, 2)
        sl = slice(g * DG, (g + 1) * DG)
        xb = x[b]
        mask = np.where(tgrid <= qidx[b][None, :], np.float32(1), np.float32(0))
        maskp = np.zeros((128, SW), dtype=np.float32)
        for i in range(NT):
            if mwid[i]:
                maskp[:, moff[i]: moff[i] + mwid[i]] = \
                    mask[128 * i: 128 * (i + 1), flo[i]: fhi[i]]
        in_maps.append(
            {
                "xT": np.ascontiguousarray(xb.T.astype(NPBF)),
                "xqT": np.ascontiguousarray(xb[qidx[b]].T.astype(NPBF)),
                "wk": np.ascontiguousarray(Wk[:, sl].astype(NPBF)),
                "wv": np.ascontiguousarray(Wv[:, sl].astype(NPBF)),
                "wq": np.ascontiguousarray(Wq[:, sl].astype(NPBF)),
                "wo": np.ascontiguousarray(Wo[sl, :].astype(NPBF)),
                "mask": np.ascontiguousarray(maskp.astype(NPBF)),
                "bks": np.ascontiguousarray(bk[sl].reshape(NHG, 128).T),
                "bqs": np.ascontiguousarray(
                    (bq[sl] / np.sqrt(HD)).reshape(NHG, 128).T.astype(np.float32)
                ),
            }
        )

    const = (bv.astype(np.float64) @ Wo.astype(np.float64) + bo).astype(np.float32)
    return flo, fhi, in_maps, const


def run(inputs, trace=False, trace_kwargs=None):
    _install_ntff_hook()
    flo, fhi, in_maps, const = _prep(inputs)
    nc = _get_program(flo, fhi)
    res = run_bass_kernel_spmd(
        nc, in_maps, list(range(8)), trace=trace, **(trace_kwargs or {})
    )
    out = np.zeros((B, M, D), dtype=np.float32)
    for b in range(B):
        out[b] = res.results[2 * b]["y"] + res.results[2 * b + 1]["y"] + const
    return out, res


def kernel(**inputs) -> np.ndarray:
    out, _ = run(inputs, trace=False)
    return out


# revision 29
# speedup vs baseline: 1.0124x; 1.0124x over previous
"""Trainium2 Bass kernel for sparse causal attention (nn_CausalAttentionKV).

Reference computation (fp32, single device):
    q_all = x @ Wq + bq ; k_all = x @ Wk + bk ; v_all = x @ Wv + bv
    q = gather(q_all, query_idx)        # (B, M, D) selected query rows
    att = softmax(mask(q k^T / sqrt(hd)))   # per-query causal mask t <= qidx[m]
    y = (att v) @ Wo + bo

Shapes: B=4, T=4096, D=2048, n_head=16, hd=128, M=512.

Sharding (8 cores): core = 2*b + g  handles batch b and head-group g
(8 heads = 1024 feature cols).  Q/K/V projections are column-parallel,
out-proj is row-parallel; the two partial outputs per batch are summed
on the host.  All matmul inputs are bf16 (fp32 PSUM accumulation).

Schedule (single fused sweep, flash-attention style): Q projection
first (d-outer so the first matmul only needs ~384KB of input), then
ONE pass over x in 512-column t-chunks.  Each t-chunk projects K and V
for all 8 heads, immediately computes the score chunks against the
resident Q^T, exps them (causal masking is applied as a 0/1 multiply
on the bf16 exp output, off the scalar engine's critical path), and
runs the P@V / row-sum matmuls lagged by ~4 head-groups so the tensor
engine never waits on the scalar-engine exp.  K/V are consumed
in-chunk: nothing is spilled to DRAM and the attention's scalar(exp)
and vector(mask/esum/accumulate) work - which made a separate
attention phase scalar-bound - hides under the 55us/chunk projection
matmul stream.  P@V partial sums and softmax denominators accumulate
in SBUF fp32 (vector adds) since PSUM (8 banks) is fully committed to
projection/score/PV pipelining.  Normalization is deferred to the end
(one bf16 broadcast matmul per head).  Input loading is split across
the sync/scalar/gpsimd DMA queues to cut the startup serialization;
the y output alternates sync/scalar queues to shrink the drain tail.
Per-chunk score skip bounds (flo/fhi) avoid fully-masked score work
(~47% of attention) exactly as in the reference semantics.
"""

import sys
import types
from contextlib import ExitStack

import numpy as np
import ml_dtypes

import concourse.bass as bass
import concourse.tile as tile
import concourse.mybir as mybir
from concourse import bacc
from concourse.bass_utils import run_bass_kernel_spmd

BF16 = mybir.dt.bfloat16
F32 = mybir.dt.float32
NPBF = ml_dtypes.bfloat16

B, T, D = 4, 4096, 2048
NH, HD, M = 16, 128, 512
NHG = 8            # heads per core (group)
DG = NHG * HD      # 1024 feature cols per core
NT = T // 128      # 32 t-chunks
ND = D // 128      # 16 d-chunks
KTS = 512          # t columns per sweep step
NTS = T // KTS     # 8 sweep steps


def _install_ntff_hook():
    """Register the axon NTFF profiling hook if the image's antenv lacks it."""
    try:
        from antenv.axon_hooks import get_axon_ntff_profile_hook  # noqa: F401
        return
    except ImportError:
        pass
    try:
        import antenv
        from trn_agent_boot.trn_boot import _ntff_profile_via_ctypes

        mod = types.ModuleType("antenv.axon_hooks")
        hook = [None]
        mod.set_axon_ntff_profile_hook = lambda h: hook.__setitem__(0, h)
        mod.get_axon_ntff_profile_hook = lambda: hook[0]
        sys.modules["antenv.axon_hooks"] = mod
        antenv.axon_hooks = mod
        mod.set_axon_ntff_profile_hook(
            _ntff_profile_via_ctypes("/opt/axon/libaxon_pjrt.so")
        )
    except Exception:
        pass


def build_program(flo, fhi):
    """Build the per-core Bass program.

    flo[i]: first m column with any allowed key in t-chunk i (cols below
            are fully masked there -> never computed).
    fhi[i]: first m column fully allowed in t-chunk i (cols beyond need
            no masking).
    Both are unions over the 4 batches so one program serves all cores.
    """
    nc = bacc.Bacc("TRN2", target_bir_lowering=False, debug=False)

    xT = nc.dram_tensor("xT", [D, T], BF16, kind="ExternalInput")
    xqT = nc.dram_tensor("xqT", [D, M], BF16, kind="ExternalInput")
    wk = nc.dram_tensor("wk", [D, DG], BF16, kind="ExternalInput")
    wv = nc.dram_tensor("wv", [D, DG], BF16, kind="ExternalInput")
    wq = nc.dram_tensor("wq", [D, DG], BF16, kind="ExternalInput")
    wo = nc.dram_tensor("wo", [DG, D], BF16, kind="ExternalInput")
    mwid = [max(fhi[i] - flo[i], 0) for i in range(NT)]
    moff = [0] * NT
    for i in range(1, NT):
        moff[i] = moff[i - 1] + mwid[i - 1]
    SW = max(moff[-1] + mwid[-1], 1)
    maskd = nc.dram_tensor("mask", [128, SW], BF16, kind="ExternalInput")
    bks = nc.dram_tensor("bks", [128, NHG], F32, kind="ExternalInput")
    bqs = nc.dram_tensor("bqs", [128, NHG], F32, kind="ExternalInput")
    y = nc.dram_tensor("y", [M, D], F32, kind="ExternalOutput")

    # (c*128+p, t) views for chunked DMA
    xTr = xT.rearrange("(c p) t -> p c t", p=128)
    xqTr = xqT.rearrange("(c p) t -> p c t", p=128)
    wkr = wk.rearrange("(c p) t -> p c t", p=128)
    wvr = wv.rearrange("(c p) t -> p c t", p=128)
    wqr = wq.rearrange("(c p) t -> p c t", p=128)
    wor = wo.rearrange("(c p) t -> p c t", p=128)

    active = [[i for i in range(4 * ts, 4 * ts + 4) if flo[i] < M]
              for ts in range(NTS)]

    with ExitStack() as ctx:
        tc = ctx.enter_context(tile.TileContext(nc))

        # ---- persistent tiles --------------------------------------
        persist = ctx.enter_context(tc.tile_pool(name="persist", bufs=1))
        qt_t = [persist.tile([128, M], BF16, name=f"qt{j}", tag=f"qt{j}") for j in range(NHG)]
        po_acc = [persist.tile([128, M], F32, name=f"po{j}", tag=f"po{j}") for j in range(NHG)]
        l_acc = [persist.tile([1, M], F32, name=f"l{j}", tag=f"l{j}") for j in range(NHG)]
        bias_k = persist.tile([128, NHG], F32, name="bias_k", tag="bias_k")
        bias_q = persist.tile([128, NHG], F32, name="bias_q", tag="bias_q")
        zbias = persist.tile([128, 1], F32, name="zbias", tag="zbias")
        ones_c = persist.tile([128, 1], BF16, name="ones_c", tag="ones_c")
        ones_r = persist.tile([1, 128], BF16, name="ones_r", tag="ones_r")
        # mask multiplicands (0/1): all chunk windows packed in one tile
        mask_t = persist.tile([128, SW], BF16, name="maskp", tag="maskp")

        nc.sync.dma_start(bias_k[:], bks[:])
        nc.sync.dma_start(bias_q[:], bqs[:])
        nc.vector.memset(zbias[:], 0.0)
        nc.vector.memset(ones_c[:], 1.0)
        nc.vector.memset(ones_r[:], 1.0)

        # K/V weights + first x chunk prefetch on the scalar HW queue;
        # masks on the gpsimd SW queue (small, needed ~55us in).
        wkp = ctx.enter_context(tc.tile_pool(name="wkp", bufs=1))
        wk_t = [wkp.tile([128, 4, DG], BF16, name=f"wk{d}", tag=f"wk{d}") for d in range(4)]
        wv_t = [wkp.tile([128, 4, DG], BF16, name=f"wv{d}", tag=f"wv{d}") for d in range(4)]
        xtp = ctx.enter_context(tc.tile_pool(name="xtp", bufs=2))
        xt0 = [xtp.tile([128, 4, KTS], BF16, name=f"xt{d}", tag=f"xt{d}") for d in range(4)]

        # ---- phase A-Q: Qt[j] = ((xq @ wq_j + bq_j)/sqrt(hd))^T ----
        # d-outer: step d needs only xq_d (128KB) + wq_d (256KB), so the
        # first matmul launches ~3us in; all 8 head PSUM groups stay open.
        inv_s = 1.0 / float(np.sqrt(HD))
        with (
            nc.named_scope("phase_AQ"),
            tc.tile_pool(name="wqp", bufs=1) as wqp,
            tc.tile_pool(name="pq", bufs=1, space="PSUM") as pqp,
        ):
            xq_t, wq_t = [], []
            for d in range(ND):
                xq_t.append(wqp.tile([128, M], BF16, name=f"xq{d}", tag=f"xq{d}"))
                wq_t.append(wqp.tile([128, DG], BF16, name=f"wq{d}", tag=f"wq{d}"))
                nc.sync.dma_start(xq_t[d][:], xqTr[:, d, :])
                (nc.sync if d % 2 == 0 else nc.scalar).dma_start(wq_t[d][:], wqr[:, d, :])
            for d in range(4):
                nc.scalar.dma_start(xt0[d][:], xTr[:, 4 * d: 4 * d + 4, 0:KTS])
            for d in range(2):
                nc.scalar.dma_start(wk_t[d][:], wkr[:, 4 * d: 4 * d + 4, :])
            for d in range(2, 4):
                nc.sync.dma_start(wk_t[d][:], wkr[:, 4 * d: 4 * d + 4, :])
            for d in range(2):
                nc.scalar.dma_start(wv_t[d][:], wvr[:, 4 * d: 4 * d + 4, :])
            for d in range(2, 4):
                nc.sync.dma_start(wv_t[d][:], wvr[:, 4 * d: 4 * d + 4, :])
            nc.sync.dma_start(mask_t[:], maskd[:])
            pq = [pqp.tile([128, M], F32, name=f"pq{j}", tag=f"pq{j}") for j in range(NHG)]
            for d in range(ND):
                for j in range(NHG):
                    nc.tensor.matmul(
                        pq[j][:],
                        wq_t[d][:, j * 128: (j + 1) * 128],
                        xq_t[d][:],
                        start=(d == 0),
                        stop=(d == ND - 1),
                        skip_group_check=True,
                    )
            for j in range(NHG):
                nc.scalar.activation(
                    qt_t[j][:], pq[j][:],
                    mybir.ActivationFunctionType.Identity,
                    scale=inv_s, bias=bias_q[:, j: j + 1],
                )

        # ---- fused sweep: K/V projection + attention per t-chunk ----
        wop = ctx.enter_context(tc.tile_pool(name="wop", bufs=1))
        wo_t = [wop.tile([128, 2, D], BF16, name="wo0", tag="wo0")]

        with (
            nc.named_scope("phase_sweep"),
            tc.tile_pool(name="kst", bufs=1) as kstp,
            tc.tile_pool(name="vst", bufs=1) as vstp,
            tc.tile_pool(name="esb", bufs=3) as esbp,
            tc.tile_pool(name="essum", bufs=1) as esump,
            tc.tile_pool(name="lsb", bufs=1) as lsbp,
            tc.tile_pool(name="pkv", bufs=2, space="PSUM") as pkvp,
            tc.tile_pool(name="ps", bufs=4, space="PSUM") as psp,
            tc.tile_pool(name="ppv", bufs=1, space="PSUM") as ppvp,
            tc.tile_pool(name="prs", bufs=1, space="PSUM") as prsp,
        ):
            pend = []      # deferred PV/rowsum jobs
            v_live = {}    # ts -> list of v tiles

            lo7 = min((flo[i] for i in active[NTS - 1]), default=0) if active[NTS - 1] else 0

            def emit_norm(j, lo, hi):
                # ot[j][:, lo:hi] = po[j] / l[j], reusing the dead qt tile
                if hi <= lo:
                    return
                linv32 = lsbp.tile([1, M], F32, name="linv32", tag=f"linv32{j % 2}")
                nc.vector.reciprocal_approx_fast(linv32[:, lo:hi], l_acc[j][:, lo:hi])
                linv = lsbp.tile([1, M], BF16, name="linv", tag=f"linv{j % 2}")
                nc.vector.tensor_copy(linv[:, lo:hi], linv32[:, lo:hi])
                pb = psp.tile([128, M], F32, name="pb", tag="s")
                nc.tensor.matmul(pb[:, lo:hi], ones_r[:], linv[:, lo:hi], start=True,
                                 stop=True, skip_group_check=True)
                nc.vector.tensor_mul(qt_t[j][:, lo:hi], po_acc[j][:, lo:hi], pb[:, lo:hi])

            def flush_pv(norm_inline=True):
                j, ts0, items = pend.pop(0)
                lo = items[0][1]
                vts = v_live[ts0]
                pvt = ppvp.tile([128, M], F32, name="pv", tag="pv")
                for k, (i, lo_i, et) in enumerate(items):
                    nc.tensor.matmul(
                        pvt[:, lo_i:M],
                        vts[i - 4 * ts0][:, j * 128: (j + 1) * 128],
                        et[:, lo_i:M],
                        start=(k == 0),
                        stop=(k == len(items) - 1),
                        skip_group_check=True,
                    )
                # esum: right-aligned sum of the chunk exps (bf16, DVE 2x)
                est = esump.tile([128, M], BF16, name="esum", tag="esum")
                nc.vector.tensor_copy(est[:, lo:M], items[0][2][:, lo:M])
                for (i, lo_i, et) in items[1:]:
                    nc.vector.tensor_add(est[:, lo_i:M], est[:, lo_i:M], et[:, lo_i:M])
                rst = prsp.tile([1, M], F32, name="rs", tag="rs")
                nc.tensor.matmul(rst[:, lo:M], ones_c[:], est[:, lo:M],
                                 start=True, stop=True, skip_group_check=True)
                if ts0 == 0:
                    nc.vector.tensor_copy(po_acc[j][:, lo:M], pvt[:, lo:M])
                    nc.vector.tensor_copy(l_acc[j][:, lo:M], rst[:, lo:M])
                else:
                    nc.vector.tensor_add(po_acc[j][:, lo:M], po_acc[j][:, lo:M], pvt[:, lo:M])
                    nc.vector.tensor_add(l_acc[j][:, lo:M], l_acc[j][:, lo:M], rst[:, lo:M])
                if ts0 == NTS - 1 and norm_inline:
                    emit_norm(j, lo7, M)
                return j, ts0

            for ts in range(NTS):
                if ts == 0:
                    xt_t = xt0
                else:
                    xt_t = [xtp.tile([128, 4, KTS], BF16, name=f"xt{d}", tag=f"xt{d}") for d in range(4)]
                    for d in range(4):
                        nc.sync.dma_start(
                            xt_t[d][:], xTr[:, 4 * d: 4 * d + 4, ts * KTS: (ts + 1) * KTS]
                        )
                if ts == NTS - 1:
                    # first wo slice rides under the last sweep step
                    nc.scalar.dma_start(wo_t[0][:], wor[:, 0:2, :])

                # K^T for all heads: kj[j] = (wk_j^T x)[hd, t]
                kj = []
                for j in range(NHG):
                    pk = pkvp.tile([128, KTS], F32, name="pk", tag="pkv")
                    for d in range(ND):
                        nc.tensor.matmul(
                            pk[:],
                            wk_t[d // 4][:, d % 4, j * 128: (j + 1) * 128],
                            xt_t[d // 4][:, d % 4, :],
                            start=(d == 0),
                            stop=(d == ND - 1),
                        )
                    kt = kstp.tile([128, KTS], BF16, name=f"k{j}", tag=f"k{j}")
                    nc.scalar.activation(
                        kt[:], pk[:],
                        mybir.ActivationFunctionType.Identity,
                        bias=bias_k[:, j: j + 1],
                    )
                    kj.append(kt)
                    if pend:
                        flush_pv()

                # V: (t, DG) tiles for this step
                vts = []
                for u in range(KTS // 128):
                    vt = vstp.tile([128, DG], BF16, name=f"v{u}", tag=f"v{u}")
                    for f in range(2):
                        pv = pkvp.tile([128, 512], F32, name="pvp", tag="pkv")
                        for d in range(ND):
                            nc.tensor.matmul(
                                pv[:],
                                xt_t[d // 4][:, d % 4, u * 128: (u + 1) * 128],
                                wv_t[d // 4][:, d % 4, f * 512: (f + 1) * 512],
                                start=(d == 0),
                                stop=(d == ND - 1),
                            )
                        nc.vector.tensor_copy(vt[:, f * 512: (f + 1) * 512], pv[:])
                    vts.append(vt)
                v_live[ts] = vts
                v_live.pop(ts - 2, None)

                if ts == NTS - 1:
                    # columns < lo7 get no contribution from this last step:
                    # normalize them now, hidden under the K/V matmul stream
                    for j in range(NHG):
                        emit_norm(j, 0, lo7)

                # scores + exp per head; PV lagged 4 head-groups
                for j in range(NHG):
                    items = []
                    for c, i in enumerate(active[ts]):
                        lo_i = flo[i]
                        st = psp.tile([128, M], F32, name="s", tag="s")
                        nc.tensor.matmul(
                            st[:, lo_i:M],
                            kj[j][:, (i % 4) * 128: (i % 4 + 1) * 128],
                            qt_t[j][:, lo_i:M],
                            start=True, stop=True, skip_group_check=True,
                        )
                        et = esbp.tile([128, M], BF16, name="e", tag=f"e{c}")
                        nc.scalar.activation(
                            et[:, lo_i:M], st[:, lo_i:M],
                            mybir.ActivationFunctionType.Exp,
                            bias=zbias[:],
                        )
                        if fhi[i] > lo_i:
                            nc.vector.tensor_mul(
                                et[:, lo_i: fhi[i]],
                                et[:, lo_i: fhi[i]],
                                mask_t[:, moff[i]: moff[i] + mwid[i]],
                            )
                        items.append((i, lo_i, et))
                    if items:
                        pend.append((j, ts, items))
                    if j >= 2 and pend:
                        flush_pv()

            # drain: flushes first so the recip chains pipeline, then norms
            tail_norms = []
            while pend:
                j, ts0 = flush_pv(norm_inline=False)
                if ts0 == NTS - 1:
                    tail_norms.append(j)
            for j in tail_norms:
                emit_norm(j, lo7, M)

        # remaining out-proj weights (space freed by the sweep pools)
        for dd in range(1, 4):
            wo_t.append(wop.tile([128, 2, D], BF16, name=f"wo{dd}", tag=f"wo{dd}"))
            nc.scalar.dma_start(wo_t[dd][:], wor[:, 2 * dd: 2 * dd + 2, :])


        # ---- phase C: y = O @ wo  (row-parallel partial) -----------
        with (
            nc.named_scope("phase_C"),
            tc.tile_pool(name="py", bufs=2, space="PSUM") as pyp,
            tc.tile_pool(name="ysb", bufs=3) as ysb,
        ):
            for mb in range(M // 128):
                for fp in range(D // 1024):
                    py = [
                        pyp.tile([128, 512], F32, name="py", tag=f"py{h}")
                        for h in range(2)
                    ]
                    for j in range(NHG):
                        for h in range(2):
                            fo = 2 * fp + h
                            nc.tensor.matmul(
                                py[h][:],
                                qt_t[j][:, mb * 128: (mb + 1) * 128],
                                wo_t[j // 2][:, j % 2, fo * 512: (fo + 1) * 512],
                                start=(j == 0),
                                stop=(j == NHG - 1),
                                skip_group_check=True,
                            )
                    for h in range(2):
                        ys = ysb.tile([128, 512], F32, name="ys", tag="ys")
                        nc.scalar.copy(ys[:], py[h][:])
                        eng = nc.sync if (2 * mb + fp + h) % 2 == 0 else nc.scalar
                        eng.dma_start(
                            y[
                                mb * 128: (mb + 1) * 128,
                                (2 * fp + h) * 512: (2 * fp + h + 1) * 512,
                            ],
                            ys[:],
                        )

    nc.compile()
    return nc


_cache = {}


def _get_program(flo, fhi):
    key = (tuple(flo), tuple(fhi))
    if key not in _cache:
        _cache[key] = build_program(list(flo), list(fhi))
    return _cache[key]


def _prep(inputs):
    x = np.asarray(inputs["x"], dtype=np.float32)
    qidx = np.asarray(inputs["query_idx"]).astype(np.int64)
    Wq = np.asarray(inputs["Wq"], dtype=np.float32)
    Wk = np.asarray(inputs["Wk"], dtype=np.float32)
    Wv = np.asarray(inputs["Wv"], dtype=np.float32)
    Wo = np.asarray(inputs["Wo"], dtype=np.float32)
    bq = np.asarray(inputs["bq"], dtype=np.float32)
    bk = np.asarray(inputs["bk"], dtype=np.float32)
    bv = np.asarray(inputs["bv"], dtype=np.float32)
    bo = np.asarray(inputs["bo"], dtype=np.float32)

    # Per-t-chunk skip bounds, union over batches.  flo[i] = first m that
    # attends into chunk i (everything below is fully masked there);
    # fhi[i] = one past the last m only partially covered by chunk i.
    # Computed positionally so they are correct even for unsorted
    # query_idx (just less effective at skipping).
    flo = [M] * NT
    fhi = [0] * NT
    for b in range(B):
        for i in range(NT):
            allowed = qidx[b] >= 128 * i          # chunk i not fully masked
            partial = qidx[b] < 128 * (i + 1)     # chunk i not fully allowed
            lo_b = int(np.argmax(allowed)) if allowed.any() else M
            hi_b = M - int(np.argmax(partial[::-1])) if partial.any() else 0
            flo[i] = min(flo[i], lo_b)
            fhi[i] = max(fhi[i], hi_b)

    # pack per-chunk mask windows [128, fhi-flo) into one [128, SW] tensor
    mwid = [max(fhi[i] - flo[i], 0) for i in range(NT)]
    moff = [0] * NT
    for i in range(1, NT):
        moff[i] = moff[i - 1] + mwid[i - 1]
    SW = max(moff[-1] + mwid[-1], 1)

    in_maps = []
    tgrid = np.arange(T)[:, None]
    for core in range(8):
        b, g = divmod(core, 2)
        sl = slice(g * DG, (g + 1) * DG)
        xb = x[b]
        mask = np.where(tgrid <= qidx[b][None, :], np.float32(1), np.float32(0))
        maskp = np.zeros((128, SW), dtype=np.float32)
        for i in range(NT):
            if mwid[i]:
                maskp[:, moff[i]: moff[i] + mwid[i]] = \
                    mask[128 * i: 128 * (i + 1), flo[i]: fhi[i]]
        in_maps.append(
            {
                "xT": np.ascontiguousarray(xb.T.astype(NPBF)),
                "xqT": np.ascontiguousarray(xb[qidx[b]].T.astype(NPBF)),
                "wk": np.ascontiguousarray(Wk[:, sl].astype(NPBF)),
                "wv": np.ascontiguousarray(Wv[:, sl].astype(NPBF)),
                "wq": np.ascontiguousarray(Wq[:, sl].astype(NPBF)),
                "wo": np.ascontiguousarray(Wo[sl, :].astype(NPBF)),
                "mask": np.ascontiguousarray(maskp.astype(NPBF)),
                "bks": np.ascontiguousarray(bk[sl].reshape(NHG, 128).T),
                "bqs": np.ascontiguousarray(
                    (bq[sl] / np.sqrt(HD)).reshape(NHG, 128).T.astype(np.float32)
                ),
            }
        )

    const = (bv.astype(np.float64) @ Wo.astype(np.float64) + bo).astype(np.float32)
    return flo, fhi, in_maps, const


def run(inputs, trace=False, trace_kwargs=None):
    _install_ntff_hook()
    flo, fhi, in_maps, const = _prep(inputs)
    nc = _get_program(flo, fhi)
    res = run_bass_kernel_spmd(
        nc, in_maps, list(range(8)), trace=trace, **(trace_kwargs or {})
    )
    out = np.zeros((B, M, D), dtype=np.float32)
    for b in range(B):
        out[b] = res.results[2 * b]["y"] + res.results[2 * b + 1]["y"] + const
    return out, res


def kernel(**inputs) -> np.ndarray:
    out, _ = run(inputs, trace=False)
    return out


# revision 31
# speedup vs baseline: 1.0178x; 1.0053x over previous
"""Trainium2 Bass kernel for sparse causal attention (nn_CausalAttentionKV).

Reference computation (fp32, single device):
    q_all = x @ Wq + bq ; k_all = x @ Wk + bk ; v_all = x @ Wv + bv
    q = gather(q_all, query_idx)        # (B, M, D) selected query rows
    att = softmax(mask(q k^T / sqrt(hd)))   # per-query causal mask t <= qidx[m]
    y = (att v) @ Wo + bo

Shapes: B=4, T=4096, D=2048, n_head=16, hd=128, M=512.

Sharding (8 cores): core = 2*b + g  handles batch b and head-group g
(8 heads = 1024 feature cols).  Q/K/V projections are column-parallel,
out-proj is row-parallel; the two partial outputs per batch are summed
on the host.  All matmul inputs are bf16 (fp32 PSUM accumulation).

Schedule (single fused sweep, flash-attention style): Q projection
first (d-outer so the first matmul only needs ~384KB of input), then
ONE pass over x in 512-column t-chunks.  Each t-chunk projects K and V
for all 8 heads, immediately computes the score chunks against the
resident Q^T, exps them (causal masking is applied as a 0/1 multiply
on the bf16 exp output, off the scalar engine's critical path), and
runs the P@V / row-sum matmuls lagged by ~4 head-groups so the tensor
engine never waits on the scalar-engine exp.  K/V are consumed
in-chunk: nothing is spilled to DRAM and the attention's scalar(exp)
and vector(mask/esum/accumulate) work - which made a separate
attention phase scalar-bound - hides under the 55us/chunk projection
matmul stream.  P@V partial sums and softmax denominators accumulate
in SBUF fp32 (vector adds) since PSUM (8 banks) is fully committed to
projection/score/PV pipelining.  Normalization is deferred to the end
(one bf16 broadcast matmul per head).  Input loading is split across
the sync/scalar/gpsimd DMA queues to cut the startup serialization;
the y output alternates sync/scalar queues to shrink the drain tail.
Per-chunk score skip bounds (flo/fhi) avoid fully-masked score work
(~47% of attention) exactly as in the reference semantics.
"""

import sys
import types
from contextlib import ExitStack

import numpy as np
import ml_dtypes

import concourse.bass as bass
import concourse.tile as tile
import concourse.mybir as mybir
from concourse import bacc
from concourse.bass_utils import run_bass_kernel_spmd

BF16 = mybir.dt.bfloat16
F32 = mybir.dt.float32
NPBF = ml_dtypes.bfloat16

B, T, D = 4, 4096, 2048
NH, HD, M = 16, 128, 512
NHG = 8            # heads per core (group)
DG = NHG * HD      # 1024 feature cols per core
NT = T // 128      # 32 t-chunks
ND = D // 128      # 16 d-chunks
KTS = 512          # t columns per sweep step
NTS = T // KTS     # 8 sweep steps


def _install_ntff_hook():
    """Register the axon NTFF profiling hook if the image's antenv lacks it."""
    try:
        from antenv.axon_hooks import get_axon_ntff_profile_hook  # noqa: F401
        return
    except ImportError:
        pass
    try:
        import antenv
        from trn_agent_boot.trn_boot import _ntff_profile_via_ctypes

        mod = types.ModuleType("antenv.axon_hooks")
        hook = [None]
        mod.set_axon_ntff_profile_hook = lambda h: hook.__setitem__(0, h)
        mod.get_axon_ntff_profile_hook = lambda: hook[0]
        sys.modules["antenv.axon_hooks"] = mod
        antenv.axon_hooks = mod
        mod.set_axon_ntff_profile_hook(
            _ntff_profile_via_ctypes("/opt/axon/libaxon_pjrt.so")
        )
    except Exception:
        pass


def build_program(flo, fhi):
    """Build the per-core Bass program.

    flo[i]: first m column with any allowed key in t-chunk i (cols below
            are fully masked there -> never computed).
    fhi[i]: first m column fully allowed in t-chunk i (cols beyond need
            no masking).
    Both are unions over the 4 batches so one program serves all cores.
    """
    nc = bacc.Bacc("TRN2", target_bir_lowering=False, debug=False)

    xT = nc.dram_tensor("xT", [D, T], BF16, kind="ExternalInput")
    xqT = nc.dram_tensor("xqT", [D, M], BF16, kind="ExternalInput")
    wk = nc.dram_tensor("wk", [D, DG], BF16, kind="ExternalInput")
    wv = nc.dram_tensor("wv", [D, DG], BF16, kind="ExternalInput")
    wq = nc.dram_tensor("wq", [D, DG], BF16, kind="ExternalInput")
    wo = nc.dram_tensor("wo", [DG, D], BF16, kind="ExternalInput")
    mwid = [max(fhi[i] - flo[i], 0) for i in range(NT)]
    moff = [0] * NT
    for i in range(1, NT):
        moff[i] = moff[i - 1] + mwid[i - 1]
    SW = max(moff[-1] + mwid[-1], 1)
    maskd = nc.dram_tensor("mask", [128, SW], BF16, kind="ExternalInput")
    bks = nc.dram_tensor("bks", [128, NHG], F32, kind="ExternalInput")
    bqs = nc.dram_tensor("bqs", [128, NHG], F32, kind="ExternalInput")
    y = nc.dram_tensor("y", [M, D], F32, kind="ExternalOutput")

    # (c*128+p, t) views for chunked DMA
    xTr = xT.rearrange("(c p) t -> p c t", p=128)
    xqTr = xqT.rearrange("(c p) t -> p c t", p=128)
    wkr = wk.rearrange("(c p) t -> p c t", p=128)
    wvr = wv.rearrange("(c p) t -> p c t", p=128)
    wqr = wq.rearrange("(c p) t -> p c t", p=128)
    wor = wo.rearrange("(c p) t -> p c t", p=128)

    active = [[i for i in range(4 * ts, 4 * ts + 4) if flo[i] < M]
              for ts in range(NTS)]

    with ExitStack() as ctx:
        tc = ctx.enter_context(tile.TileContext(nc))

        # ---- persistent tiles --------------------------------------
        persist = ctx.enter_context(tc.tile_pool(name="persist", bufs=1))
        qt_t = [persist.tile([128, M], BF16, name=f"qt{j}", tag=f"qt{j}") for j in range(NHG)]
        po_acc = [persist.tile([128, M], F32, name=f"po{j}", tag=f"po{j}") for j in range(NHG)]
        l_acc = [persist.tile([1, M], F32, name=f"l{j}", tag=f"l{j}") for j in range(NHG)]
        bias_k = persist.tile([128, NHG], F32, name="bias_k", tag="bias_k")
        bias_q = persist.tile([128, NHG], F32, name="bias_q", tag="bias_q")
        zbias = persist.tile([128, 1], F32, name="zbias", tag="zbias")
        ones_c = persist.tile([128, 1], BF16, name="ones_c", tag="ones_c")
        ones_r = persist.tile([1, 128], BF16, name="ones_r", tag="ones_r")
        # mask multiplicands (0/1): all chunk windows packed in one tile
        mask_t = persist.tile([128, SW], BF16, name="maskp", tag="maskp")

        nc.sync.dma_start(bias_k[:], bks[:])
        nc.sync.dma_start(bias_q[:], bqs[:])
        nc.vector.memset(zbias[:], 0.0)
        nc.vector.memset(ones_c[:], 1.0)
        nc.vector.memset(ones_r[:], 1.0)

        # K/V weights + first x chunk prefetch on the scalar HW queue;
        # masks on the gpsimd SW queue (small, needed ~55us in).
        wkp = ctx.enter_context(tc.tile_pool(name="wkp", bufs=1))
        wk_t = [wkp.tile([128, 4, DG], BF16, name=f"wk{d}", tag=f"wk{d}") for d in range(4)]
        wv_t = [wkp.tile([128, 4, DG], BF16, name=f"wv{d}", tag=f"wv{d}") for d in range(4)]
        xtp = ctx.enter_context(tc.tile_pool(name="xtp", bufs=2))
        xt0 = [xtp.tile([128, 4, KTS], BF16, name=f"xt{d}", tag=f"xt{d}") for d in range(4)]

        # ---- phase A-Q: Qt[j] = ((xq @ wq_j + bq_j)/sqrt(hd))^T ----
        # d-outer: step d needs only xq_d (128KB) + wq_d (256KB), so the
        # first matmul launches ~3us in; all 8 head PSUM groups stay open.
        inv_s = 1.0 / float(np.sqrt(HD))
        with (
            nc.named_scope("phase_AQ"),
            tc.tile_pool(name="wqp", bufs=1) as wqp,
            tc.tile_pool(name="pq", bufs=1, space="PSUM") as pqp,
        ):
            xq_t, wq_t = [], []
            for d in range(ND):
                xq_t.append(wqp.tile([128, M], BF16, name=f"xq{d}", tag=f"xq{d}"))
                wq_t.append(wqp.tile([128, DG], BF16, name=f"wq{d}", tag=f"wq{d}"))
                nc.sync.dma_start(xq_t[d][:], xqTr[:, d, :])
                (nc.sync if d % 2 == 0 else nc.scalar).dma_start(wq_t[d][:], wqr[:, d, :])
            for d in range(4):
                nc.scalar.dma_start(xt0[d][:], xTr[:, 4 * d: 4 * d + 4, 0:KTS])
            for d in range(2):
                nc.scalar.dma_start(wk_t[d][:], wkr[:, 4 * d: 4 * d + 4, :])
            for d in range(2, 4):
                nc.sync.dma_start(wk_t[d][:], wkr[:, 4 * d: 4 * d + 4, :])
            for d in range(2):
                nc.scalar.dma_start(wv_t[d][:], wvr[:, 4 * d: 4 * d + 4, :])
            for d in range(2, 4):
                nc.sync.dma_start(wv_t[d][:], wvr[:, 4 * d: 4 * d + 4, :])
            nc.sync.dma_start(mask_t[:], maskd[:])
            pq = [pqp.tile([128, M], F32, name=f"pq{j}", tag=f"pq{j}") for j in range(NHG)]
            for d in range(ND):
                for j in range(NHG):
                    nc.tensor.matmul(
                        pq[j][:],
                        wq_t[d][:, j * 128: (j + 1) * 128],
                        xq_t[d][:],
                        start=(d == 0),
                        stop=(d == ND - 1),
                        skip_group_check=True,
                    )
            for j in range(NHG):
                nc.scalar.activation(
                    qt_t[j][:], pq[j][:],
                    mybir.ActivationFunctionType.Identity,
                    scale=inv_s, bias=bias_q[:, j: j + 1],
                )

        # ---- fused sweep: K/V projection + attention per t-chunk ----
        wop = ctx.enter_context(tc.tile_pool(name="wop", bufs=1))
        wo_t = [wop.tile([128, 2, D], BF16, name="wo0", tag="wo0")]

        with (
            nc.named_scope("phase_sweep"),
            tc.tile_pool(name="kst", bufs=1) as kstp,
            tc.tile_pool(name="vst", bufs=1) as vstp,
            tc.tile_pool(name="esb", bufs=3) as esbp,
            tc.tile_pool(name="essum", bufs=1) as esump,
            tc.tile_pool(name="lsb", bufs=1) as lsbp,
            tc.tile_pool(name="pkv", bufs=2, space="PSUM") as pkvp,
            tc.tile_pool(name="ps", bufs=4, space="PSUM") as psp,
            tc.tile_pool(name="ppv", bufs=1, space="PSUM") as ppvp,
            tc.tile_pool(name="prs", bufs=1, space="PSUM") as prsp,
        ):
            pend = []      # deferred PV/rowsum jobs
            v_live = {}    # ts -> list of v tiles

            lo7 = min((flo[i] for i in active[NTS - 1]), default=0) if active[NTS - 1] else 0

            def emit_norm(j, lo, hi):
                # ot[j][:, lo:hi] = po[j] / l[j], reusing the dead qt tile
                if hi <= lo:
                    return
                linv32 = lsbp.tile([1, M], F32, name="linv32", tag=f"linv32{j % 2}")
                nc.vector.reciprocal_approx_fast(linv32[:, lo:hi], l_acc[j][:, lo:hi])
                linv = lsbp.tile([1, M], BF16, name="linv", tag=f"linv{j % 2}")
                nc.vector.tensor_copy(linv[:, lo:hi], linv32[:, lo:hi])
                pb = psp.tile([128, M], F32, name="pb", tag="s")
                nc.tensor.matmul(pb[:, lo:hi], ones_r[:], linv[:, lo:hi], start=True,
                                 stop=True, skip_group_check=True)
                nc.vector.tensor_mul(qt_t[j][:, lo:hi], po_acc[j][:, lo:hi], pb[:, lo:hi])

            def flush_pv(norm_inline=True):
                j, ts0, items = pend.pop(0)
                lo = items[0][1]
                vts = v_live[ts0]
                pvt = ppvp.tile([128, M], F32, name="pv", tag="pv")
                for k, (i, lo_i, et) in enumerate(items):
                    nc.tensor.matmul(
                        pvt[:, lo_i:M],
                        vts[i - 4 * ts0][:, j * 128: (j + 1) * 128],
                        et[:, lo_i:M],
                        start=(k == 0),
                        stop=(k == len(items) - 1),
                        skip_group_check=True,
                    )
                # esum: right-aligned sum of the chunk exps (bf16, DVE 2x)
                est = esump.tile([128, M], BF16, name="esum", tag="esum")
                nc.vector.tensor_copy(est[:, lo:M], items[0][2][:, lo:M])
                for (i, lo_i, et) in items[1:]:
                    nc.vector.tensor_add(est[:, lo_i:M], est[:, lo_i:M], et[:, lo_i:M])
                rst = prsp.tile([1, M], F32, name="rs", tag="rs")
                nc.tensor.matmul(rst[:, lo:M], ones_c[:], est[:, lo:M],
                                 start=True, stop=True, skip_group_check=True)
                if ts0 == 0:
                    nc.vector.tensor_copy(po_acc[j][:, lo:M], pvt[:, lo:M])
                    nc.vector.tensor_copy(l_acc[j][:, lo:M], rst[:, lo:M])
                else:
                    nc.vector.tensor_add(po_acc[j][:, lo:M], po_acc[j][:, lo:M], pvt[:, lo:M])
                    nc.vector.tensor_add(l_acc[j][:, lo:M], l_acc[j][:, lo:M], rst[:, lo:M])
                if ts0 == NTS - 1 and norm_inline:
                    emit_norm(j, lo7, M)
                return j, ts0

            for ts in range(NTS):
                if ts == 0:
                    xt_t = xt0
                else:
                    xt_t = [xtp.tile([128, 4, KTS], BF16, name=f"xt{d}", tag=f"xt{d}") for d in range(4)]
                    for d in range(4):
                        nc.sync.dma_start(
                            xt_t[d][:], xTr[:, 4 * d: 4 * d + 4, ts * KTS: (ts + 1) * KTS]
                        )
                if ts == NTS - 1:
                    # first wo slice rides under the last sweep step
                    nc.scalar.dma_start(wo_t[0][:], wor[:, 0:2, :])

                # K^T for all heads: kj[j] = (wk_j^T x)[hd, t]
                kj = []
                for j in range(NHG):
                    pk = pkvp.tile([128, KTS], F32, name="pk", tag="pkv")
                    for d in range(ND):
                        nc.tensor.matmul(
                            pk[:],
                            wk_t[d // 4][:, d % 4, j * 128: (j + 1) * 128],
                            xt_t[d // 4][:, d % 4, :],
                            start=(d == 0),
                            stop=(d == ND - 1),
                        )
                    kt = kstp.tile([128, KTS], BF16, name=f"k{j}", tag=f"k{j}")
                    nc.scalar.activation(
                        kt[:], pk[:],
                        mybir.ActivationFunctionType.Identity,
                        bias=bias_k[:, j: j + 1],
                    )
                    kj.append(kt)
                    if pend:
                        flush_pv()
                    if ts == NTS - 1 and j >= 2:
                        # cols < lo7 are final: normalize them spread across
                        # the K stream (light vector queue -> short recips)
                        emit_norm(j - 2, 0, lo7)

                # V: (t, DG) tiles for this step
                vts = []
                for u in range(KTS // 128):
                    vt = vstp.tile([128, DG], BF16, name=f"v{u}", tag=f"v{u}")
                    for f in range(2):
                        pv = pkvp.tile([128, 512], F32, name="pvp", tag="pkv")
                        for d in range(ND):
                            nc.tensor.matmul(
                                pv[:],
                                xt_t[d // 4][:, d % 4, u * 128: (u + 1) * 128],
                                wv_t[d // 4][:, d % 4, f * 512: (f + 1) * 512],
                                start=(d == 0),
                                stop=(d == ND - 1),
                            )
                        nc.vector.tensor_copy(vt[:, f * 512: (f + 1) * 512], pv[:])
                    vts.append(vt)
                    if ts == NTS - 1 and u in (1, 3):
                        emit_norm(6 + (u - 1) // 2, 0, lo7)
                v_live[ts] = vts
                v_live.pop(ts - 2, None)

                # scores + exp per head; PV lagged 4 head-groups
                for j in range(NHG):
                    items = []
                    for c, i in enumerate(active[ts]):
                        lo_i = flo[i]
                        st = psp.tile([128, M], F32, name="s", tag="s")
                        nc.tensor.matmul(
                            st[:, lo_i:M],
                            kj[j][:, (i % 4) * 128: (i % 4 + 1) * 128],
                            qt_t[j][:, lo_i:M],
                            start=True, stop=True, skip_group_check=True,
                        )
                        et = esbp.tile([128, M], BF16, name="e", tag=f"e{c}")
                        nc.scalar.activation(
                            et[:, lo_i:M], st[:, lo_i:M],
                            mybir.ActivationFunctionType.Exp,
                            bias=zbias[:],
                        )
                        if fhi[i] > lo_i:
                            nc.vector.tensor_mul(
                                et[:, lo_i: fhi[i]],
                                et[:, lo_i: fhi[i]],
                                mask_t[:, moff[i]: moff[i] + mwid[i]],
                            )
                        items.append((i, lo_i, et))
                    if items:
                        pend.append((j, ts, items))
                    if j >= 2 and pend:
                        flush_pv()

            # drain: flushes first so the recip chains pipeline, then norms
            tail_norms = []
            while pend:
                j, ts0 = flush_pv(norm_inline=False)
                if ts0 == NTS - 1:
                    tail_norms.append(j)
            for j in tail_norms:
                emit_norm(j, lo7, M)

        # remaining out-proj weights (space freed by the sweep pools)
        for dd in range(1, 4):
            wo_t.append(wop.tile([128, 2, D], BF16, name=f"wo{dd}", tag=f"wo{dd}"))
            nc.scalar.dma_start(wo_t[dd][:], wor[:, 2 * dd: 2 * dd + 2, :])


        # ---- phase C: y = O @ wo  (row-parallel partial) -----------
        with (
            nc.named_scope("phase_C"),
            tc.tile_pool(name="py", bufs=2, space="PSUM") as pyp,
            tc.tile_pool(name="ysb", bufs=3) as ysb,
        ):
            for mb in range(M // 128):
                for fp in range(D // 1024):
                    py = [
                        pyp.tile([128, 512], F32, name="py", tag=f"py{h}")
                        for h in range(2)
                    ]
                    for j in range(NHG):
                        for h in range(2):
                            fo = 2 * fp + h
                            nc.tensor.matmul(
                                py[h][:],
                                qt_t[j][:, mb * 128: (mb + 1) * 128],
                                wo_t[j // 2][:, j % 2, fo * 512: (fo + 1) * 512],
                                start=(j == 0),
                                stop=(j == NHG - 1),
                                skip_group_check=True,
                            )
                    for h in range(2):
                        ys = ysb.tile([128, 512], F32, name="ys", tag="ys")
                        nc.scalar.copy(ys[:], py[h][:])
                        eng = nc.sync if (2 * mb + fp + h) % 2 == 0 else nc.scalar
                        eng.dma_start(
                            y[
                                mb * 128: (mb + 1) * 128,
                                (2 * fp + h) * 512: (2 * fp + h + 1) * 512,
                            ],
                            ys[:],
                        )

    nc.compile()
    return nc


_cache = {}


def _get_program(flo, fhi):
    key = (tuple(flo), tuple(fhi))
    if key not in _cache:
        _cache[key] = build_program(list(flo), list(fhi))
    return _cache[key]


def _prep(inputs):
    x = np.asarray(inputs["x"], dtype=np.float32)
    qidx = np.asarray(inputs["query_idx"]).astype(np.int64)
    Wq = np.asarray(inputs["Wq"], dtype=np.float32)
    Wk = np.asarray(inputs["Wk"], dtype=np.float32)
    Wv = np.asarray(inputs["Wv"], dtype=np.float32)
    Wo = np.asarray(inputs["Wo"], dtype=np.float32)
    bq = np.asarray(inputs["bq"], dtype=np.float32)
    bk = np.asarray(inputs["bk"], dtype=np.float32)
    bv = np.asarray(inputs["bv"], dtype=np.float32)
    bo = np.asarray(inputs["bo"], dtype=np.float32)

    # Per-t-chunk skip bounds, union over batches.  flo[i] = first m that
    # attends into chunk i (everything below is fully masked there);
    # fhi[i] = one past the last m only partially covered by chunk i.
    # Computed positionally so they are correct even for unsorted
    # query_idx (just less effective at skipping).
    flo = [M] * NT
    fhi = [0] * NT
    for b in range(B):
        for i in range(NT):
            allowed = qidx[b] >= 128 * i          # chunk i not fully masked
            partial = qidx[b] < 128 * (i + 1)     # chunk i not fully allowed
            lo_b = int(np.argmax(allowed)) if allowed.any() else M
            hi_b = M - int(np.argmax(partial[::-1])) if partial.any() else 0
            flo[i] = min(flo[i], lo_b)
            fhi[i] = max(fhi[i], hi_b)

    # pack per-chunk mask windows [128, fhi-flo) into one [128, SW] tensor
    mwid = [max(fhi[i] - flo[i], 0) for i in range(NT)]
    moff = [0] * NT
    for i in range(1, NT):
        moff[i] = moff[i - 1] + mwid[i - 1]
    SW = max(moff[-1] + mwid[-1], 1)

    in_maps = []
    tgrid = np.arange(T)[:, None]
    for core in range(8):
        b, g = divmod(core, 2)
        sl = slice(g * DG, (g + 1) * DG)
        xb = x[b]
        mask = np.where(tgrid <= qidx[b][None, :], np.float32(1), np.float32(0))
        maskp = np.zeros((128, SW), dtype=np.float32)
        for i in range(NT):
            if mwid[i]:
                maskp[:, moff[i]: moff[i] + mwid[i]] = \
                    mask[128 * i: 128 * (i + 1), flo[i]: fhi[i]]
        in_maps.append(
            {
                "xT": np.ascontiguousarray(xb.T.astype(NPBF)),
                "xqT": np.ascontiguousarray(xb[qidx[b]].T.astype(NPBF)),
                "wk": np.ascontiguousarray(Wk[:, sl].astype(NPBF)),
                "wv": np.ascontiguousarray(Wv[:, sl].astype(NPBF)),
                "wq": np.ascontiguousarray(Wq[:, sl].astype(NPBF)),
                "wo": np.ascontiguousarray(Wo[sl, :].astype(NPBF)),
                "mask": np.ascontiguousarray(maskp.astype(NPBF)),
                "bks": np.ascontiguousarray(bk[sl].reshape(NHG, 128).T),
                "bqs": np.ascontiguousarray(
                    (bq[sl] / np.sqrt(HD)).reshape(NHG, 128).T.astype(np.float32)
                ),
            }
        )

    const = (bv.astype(np.float64) @ Wo.astype(np.float64) + bo).astype(np.float32)
    return flo, fhi, in_maps, const


def run(inputs, trace=False, trace_kwargs=None):
    _install_ntff_hook()
    flo, fhi, in_maps, const = _prep(inputs)
    nc = _get_program(flo, fhi)
    res = run_bass_kernel_spmd(
        nc, in_maps, list(range(8)), trace=trace, **(trace_kwargs or {})
    )
    out = np.zeros((B, M, D), dtype=np.float32)
    for b in range(B):
        out[b] = res.results[2 * b]["y"] + res.results[2 * b + 1]["y"] + const
    return out, res


def kernel(**inputs) -> np.ndarray:
    out, _ = run(inputs, trace=False)
    return out


# revision 36
# speedup vs baseline: 1.0185x; 1.0007x over previous
"""Trainium2 Bass kernel for sparse causal attention (nn_CausalAttentionKV).

Reference computation (fp32, single device):
    q_all = x @ Wq + bq ; k_all = x @ Wk + bk ; v_all = x @ Wv + bv
    q = gather(q_all, query_idx)        # (B, M, D) selected query rows
    att = softmax(mask(q k^T / sqrt(hd)))   # per-query causal mask t <= qidx[m]
    y = (att v) @ Wo + bo

Shapes: B=4, T=4096, D=2048, n_head=16, hd=128, M=512.

Sharding (8 cores): core = 2*b + g  handles batch b and head-group g
(8 heads = 1024 feature cols).  Q/K/V projections are column-parallel,
out-proj is row-parallel; the two partial outputs per batch are summed
on the host.  All matmul inputs are bf16 (fp32 PSUM accumulation).

Schedule (single fused sweep, flash-attention style): Q projection
first (d-outer so the first matmul only needs ~384KB of input), then
ONE pass over x in 512-column t-chunks.  Each t-chunk projects K and V
for all 8 heads, immediately computes the score chunks against the
resident Q^T, exps them (causal masking is applied as a 0/1 multiply
on the bf16 exp output, off the scalar engine's critical path), and
runs the P@V / row-sum matmuls lagged by 2+ head-groups so the tensor
engine never waits on the scalar-engine exp.  K/V are consumed
in-chunk: nothing is spilled to DRAM and the attention's scalar(exp)
and vector(mask/esum/accumulate) work - which made a separate
attention phase scalar-bound - hides under the 55us/chunk projection
matmul stream.  P@V partial sums and softmax denominators accumulate
in SBUF fp32 (vector adds) since PSUM (8 banks) is fully committed to
projection/score/PV pipelining.  Normalization is deferred to the end
(one bf16 broadcast matmul per head; columns finalized before the
last sweep step are normalized early, hidden under its K stream).
Input loading is split across the sync and scalar HW DMA queues to cut
the startup serialization (the ~12MB critical set is at the per-core
HBM bandwidth floor); the y output alternates both queues to shrink
the drain tail.
Per-chunk score skip bounds (flo/fhi) avoid fully-masked score work
(~47% of attention) exactly as in the reference semantics.
"""

import sys
import types
from contextlib import ExitStack

import numpy as np
import ml_dtypes

import concourse.bass as bass
import concourse.tile as tile
import concourse.mybir as mybir
from concourse import bacc
from concourse.bass_utils import run_bass_kernel_spmd

BF16 = mybir.dt.bfloat16
F32 = mybir.dt.float32
NPBF = ml_dtypes.bfloat16

B, T, D = 4, 4096, 2048
NH, HD, M = 16, 128, 512
NHG = 8            # heads per core (group)
DG = NHG * HD      # 1024 feature cols per core
NT = T // 128      # 32 t-chunks
ND = D // 128      # 16 d-chunks
KTS = 512          # t columns per sweep step
NTS = T // KTS     # 8 sweep steps


def _install_ntff_hook():
    """Register the axon NTFF profiling hook if the image's antenv lacks it."""
    try:
        from antenv.axon_hooks import get_axon_ntff_profile_hook  # noqa: F401
        return
    except ImportError:
        pass
    try:
        import antenv
        from trn_agent_boot.trn_boot import _ntff_profile_via_ctypes

        mod = types.ModuleType("antenv.axon_hooks")
        hook = [None]
        mod.set_axon_ntff_profile_hook = lambda h: hook.__setitem__(0, h)
        mod.get_axon_ntff_profile_hook = lambda: hook[0]
        sys.modules["antenv.axon_hooks"] = mod
        antenv.axon_hooks = mod
        mod.set_axon_ntff_profile_hook(
            _ntff_profile_via_ctypes("/opt/axon/libaxon_pjrt.so")
        )
    except Exception:
        pass


def build_program(flo, fhi):
    """Build the per-core Bass program.

    flo[i]: first m column with any allowed key in t-chunk i (cols below
            are fully masked there -> never computed).
    fhi[i]: first m column fully allowed in t-chunk i (cols beyond need
            no masking).
    Both are unions over the 4 batches so one program serves all cores.
    """
    nc = bacc.Bacc("TRN2", target_bir_lowering=False, debug=False)

    xT = nc.dram_tensor("xT", [D, T], BF16, kind="ExternalInput")
    xqT = nc.dram_tensor("xqT", [D, M], BF16, kind="ExternalInput")
    wk = nc.dram_tensor("wk", [D, DG], BF16, kind="ExternalInput")
    wv = nc.dram_tensor("wv", [D, DG], BF16, kind="ExternalInput")
    wq = nc.dram_tensor("wq", [D, DG], BF16, kind="ExternalInput")
    wo = nc.dram_tensor("wo", [DG, D], BF16, kind="ExternalInput")
    mwid = [max(fhi[i] - flo[i], 0) for i in range(NT)]
    moff = [0] * NT
    for i in range(1, NT):
        moff[i] = moff[i - 1] + mwid[i - 1]
    SW = max(moff[-1] + mwid[-1], 1)
    maskd = nc.dram_tensor("mask", [128, SW], BF16, kind="ExternalInput")
    bks = nc.dram_tensor("bks", [128, NHG], F32, kind="ExternalInput")
    bqs = nc.dram_tensor("bqs", [128, NHG], F32, kind="ExternalInput")
    y = nc.dram_tensor("y", [M, D], F32, kind="ExternalOutput")

    # (c*128+p, t) views for chunked DMA
    xTr = xT.rearrange("(c p) t -> p c t", p=128)
    xqTr = xqT.rearrange("(c p) t -> p c t", p=128)
    wkr = wk.rearrange("(c p) t -> p c t", p=128)
    wvr = wv.rearrange("(c p) t -> p c t", p=128)
    wqr = wq.rearrange("(c p) t -> p c t", p=128)
    wor = wo.rearrange("(c p) t -> p c t", p=128)

    active = [[i for i in range(4 * ts, 4 * ts + 4) if flo[i] < M]
              for ts in range(NTS)]

    with ExitStack() as ctx:
        tc = ctx.enter_context(tile.TileContext(nc))

        # ---- persistent tiles --------------------------------------
        persist = ctx.enter_context(tc.tile_pool(name="persist", bufs=1))
        qt_t = [persist.tile([128, M], BF16, name=f"qt{j}", tag=f"qt{j}") for j in range(NHG)]
        po_acc = [persist.tile([128, M], F32, name=f"po{j}", tag=f"po{j}") for j in range(NHG)]
        l_acc = [persist.tile([1, M], F32, name=f"l{j}", tag=f"l{j}") for j in range(NHG)]
        bias_k = persist.tile([128, NHG], F32, name="bias_k", tag="bias_k")
        bias_q = persist.tile([128, NHG], F32, name="bias_q", tag="bias_q")
        zbias = persist.tile([128, 1], F32, name="zbias", tag="zbias")
        ones_c = persist.tile([128, 1], BF16, name="ones_c", tag="ones_c")
        ones_r = persist.tile([1, 128], BF16, name="ones_r", tag="ones_r")
        # mask multiplicands (0/1): all chunk windows packed in one tile
        mask_t = persist.tile([128, SW], BF16, name="maskp", tag="maskp")

        nc.sync.dma_start(bias_k[:], bks[:])
        nc.sync.dma_start(bias_q[:], bqs[:])
        nc.vector.memset(zbias[:], 0.0)
        nc.vector.memset(ones_c[:], 1.0)
        nc.vector.memset(ones_r[:], 1.0)

        # K/V weights + first x chunk prefetch on the scalar HW queue;
        # masks on the gpsimd SW queue (small, needed ~55us in).
        wkp = ctx.enter_context(tc.tile_pool(name="wkp", bufs=1))
        wk_t = [wkp.tile([128, 4, DG], BF16, name=f"wk{d}", tag=f"wk{d}") for d in range(4)]
        wv_t = [wkp.tile([128, 4, DG], BF16, name=f"wv{d}", tag=f"wv{d}") for d in range(4)]
        xtp = ctx.enter_context(tc.tile_pool(name="xtp", bufs=2))
        xt0 = [xtp.tile([128, 4, KTS], BF16, name=f"xt{d}", tag=f"xt{d}") for d in range(4)]

        # ---- phase A-Q: Qt[j] = ((xq @ wq_j + bq_j)/sqrt(hd))^T ----
        # d-outer: step d needs only xq_d (128KB) + wq_d (256KB), so the
        # first matmul launches ~3us in; all 8 head PSUM groups stay open.
        inv_s = 1.0 / float(np.sqrt(HD))
        with (
            nc.named_scope("phase_AQ"),
            tc.tile_pool(name="wqp", bufs=1) as wqp,
            tc.tile_pool(name="pq", bufs=1, space="PSUM") as pqp,
        ):
            xq_t, wq_t = [], []
            for d in range(ND):
                xq_t.append(wqp.tile([128, M], BF16, name=f"xq{d}", tag=f"xq{d}"))
                wq_t.append(wqp.tile([128, DG], BF16, name=f"wq{d}", tag=f"wq{d}"))
                nc.sync.dma_start(xq_t[d][:], xqTr[:, d, :])
                (nc.sync if d % 2 == 0 else nc.scalar).dma_start(wq_t[d][:], wqr[:, d, :])
            for d in range(4):
                nc.scalar.dma_start(xt0[d][:], xTr[:, 4 * d: 4 * d + 4, 0:KTS])
            for d in range(2):
                nc.scalar.dma_start(wk_t[d][:], wkr[:, 4 * d: 4 * d + 4, :])
            for d in range(2, 4):
                nc.sync.dma_start(wk_t[d][:], wkr[:, 4 * d: 4 * d + 4, :])
            for d in range(2):
                nc.scalar.dma_start(wv_t[d][:], wvr[:, 4 * d: 4 * d + 4, :])
            for d in range(2, 4):
                nc.sync.dma_start(wv_t[d][:], wvr[:, 4 * d: 4 * d + 4, :])
            nc.sync.dma_start(mask_t[:], maskd[:])
            pq = [pqp.tile([128, M], F32, name=f"pq{j}", tag=f"pq{j}") for j in range(NHG)]
            for d in range(ND):
                for j in range(NHG):
                    nc.tensor.matmul(
                        pq[j][:],
                        wq_t[d][:, j * 128: (j + 1) * 128],
                        xq_t[d][:],
                        start=(d == 0),
                        stop=(d == ND - 1),
                        skip_group_check=True,
                    )
            for j in range(NHG):
                nc.scalar.activation(
                    qt_t[j][:], pq[j][:],
                    mybir.ActivationFunctionType.Identity,
                    scale=inv_s, bias=bias_q[:, j: j + 1],
                )

        # ---- fused sweep: K/V projection + attention per t-chunk ----
        wop = ctx.enter_context(tc.tile_pool(name="wop", bufs=1))
        wo_t = [wop.tile([128, 2, D], BF16, name="wo0", tag="wo0")]

        with (
            nc.named_scope("phase_sweep"),
            tc.tile_pool(name="kst", bufs=1) as kstp,
            tc.tile_pool(name="vst", bufs=1) as vstp,
            tc.tile_pool(name="esb", bufs=3) as esbp,
            tc.tile_pool(name="essum", bufs=1) as esump,
            tc.tile_pool(name="lsb", bufs=1) as lsbp,
            tc.tile_pool(name="pkv", bufs=2, space="PSUM") as pkvp,
            tc.tile_pool(name="ps", bufs=4, space="PSUM") as psp,
            tc.tile_pool(name="ppv", bufs=1, space="PSUM") as ppvp,
            tc.tile_pool(name="prs", bufs=1, space="PSUM") as prsp,
        ):
            pend = []      # deferred PV/rowsum jobs
            v_live = {}    # ts -> list of v tiles

            lo7 = min((flo[i] for i in active[NTS - 1]), default=0) if active[NTS - 1] else 0

            def emit_norm(j, lo, hi):
                # ot[j][:, lo:hi] = po[j] / l[j], reusing the dead qt tile
                if hi <= lo:
                    return
                linv32 = lsbp.tile([1, M], F32, name="linv32", tag=f"linv32{j % 2}")
                nc.vector.reciprocal_approx_fast(linv32[:, lo:hi], l_acc[j][:, lo:hi])
                linv = lsbp.tile([1, M], BF16, name="linv", tag=f"linv{j % 2}")
                nc.vector.tensor_copy(linv[:, lo:hi], linv32[:, lo:hi])
                pb = psp.tile([128, M], F32, name="pb", tag="s")
                nc.tensor.matmul(pb[:, lo:hi], ones_r[:], linv[:, lo:hi], start=True,
                                 stop=True, skip_group_check=True)
                nc.vector.tensor_mul(qt_t[j][:, lo:hi], po_acc[j][:, lo:hi], pb[:, lo:hi])

            def flush_pv(norm_inline=True):
                j, ts0, items = pend.pop(0)
                lo = items[0][1]
                vts = v_live[ts0]
                pvt = ppvp.tile([128, M], F32, name="pv", tag="pv")
                for k, (i, lo_i, et) in enumerate(items):
                    nc.tensor.matmul(
                        pvt[:, lo_i:M],
                        vts[i - 4 * ts0][:, j * 128: (j + 1) * 128],
                        et[:, lo_i:M],
                        start=(k == 0),
                        stop=(k == len(items) - 1),
                        skip_group_check=True,
                    )
                # esum: right-aligned sum of the chunk exps (bf16, DVE 2x)
                est = esump.tile([128, M], BF16, name="esum", tag="esum")
                nc.vector.tensor_copy(est[:, lo:M], items[0][2][:, lo:M])
                for (i, lo_i, et) in items[1:]:
                    nc.vector.tensor_add(est[:, lo_i:M], est[:, lo_i:M], et[:, lo_i:M])
                rst = prsp.tile([1, M], F32, name="rs", tag="rs")
                nc.tensor.matmul(rst[:, lo:M], ones_c[:], est[:, lo:M],
                                 start=True, stop=True, skip_group_check=True)
                if ts0 == 0:
                    nc.vector.tensor_copy(po_acc[j][:, lo:M], pvt[:, lo:M])
                    nc.vector.tensor_copy(l_acc[j][:, lo:M], rst[:, lo:M])
                else:
                    nc.vector.tensor_add(po_acc[j][:, lo:M], po_acc[j][:, lo:M], pvt[:, lo:M])
                    nc.vector.tensor_add(l_acc[j][:, lo:M], l_acc[j][:, lo:M], rst[:, lo:M])
                if ts0 == NTS - 1 and norm_inline:
                    emit_norm(j, lo7, M)
                return j, ts0

            for ts in range(NTS):
                if ts == 0:
                    xt_t = xt0
                else:
                    xt_t = [xtp.tile([128, 4, KTS], BF16, name=f"xt{d}", tag=f"xt{d}") for d in range(4)]
                    for d in range(4):
                        nc.sync.dma_start(
                            xt_t[d][:], xTr[:, 4 * d: 4 * d + 4, ts * KTS: (ts + 1) * KTS]
                        )
                if ts == NTS - 1:
                    # first wo slice rides under the last sweep step
                    nc.scalar.dma_start(wo_t[0][:], wor[:, 0:2, :])

                # K^T for all heads: kj[j] = (wk_j^T x)[hd, t]
                kj = []
                for j in range(NHG):
                    pk = pkvp.tile([128, KTS], F32, name="pk", tag="pkv")
                    for d in range(ND):
                        nc.tensor.matmul(
                            pk[:],
                            wk_t[d // 4][:, d % 4, j * 128: (j + 1) * 128],
                            xt_t[d // 4][:, d % 4, :],
                            start=(d == 0),
                            stop=(d == ND - 1),
                        )
                    kt = kstp.tile([128, KTS], BF16, name=f"k{j}", tag=f"k{j}")
                    nc.scalar.activation(
                        kt[:], pk[:],
                        mybir.ActivationFunctionType.Identity,
                        bias=bias_k[:, j: j + 1],
                    )
                    kj.append(kt)
                    if pend:
                        flush_pv()
                    if ts == NTS - 1 and j >= 2:
                        # cols < lo7 are final: normalize them spread across
                        # the K stream (light vector queue -> short recips)
                        emit_norm(j - 2, 0, lo7)

                # V: (t, DG) tiles for this step
                vts = []
                for u in range(KTS // 128):
                    vt = vstp.tile([128, DG], BF16, name=f"v{u}", tag=f"v{u}")
                    for f in range(2):
                        pv = pkvp.tile([128, 512], F32, name="pvp", tag="pkv")
                        for d in range(ND):
                            nc.tensor.matmul(
                                pv[:],
                                xt_t[d // 4][:, d % 4, u * 128: (u + 1) * 128],
                                wv_t[d // 4][:, d % 4, f * 512: (f + 1) * 512],
                                start=(d == 0),
                                stop=(d == ND - 1),
                            )
                        nc.vector.tensor_copy(vt[:, f * 512: (f + 1) * 512], pv[:])
                    vts.append(vt)
                    if ts == NTS - 1 and u in (1, 3):
                        emit_norm(6 + (u - 1) // 2, 0, lo7)
                v_live[ts] = vts
                v_live.pop(ts - 2, None)

                # scores + exp per head; PV lagged 4 head-groups
                for j in range(NHG):
                    items = []
                    for c, i in enumerate(active[ts]):
                        lo_i = flo[i]
                        st = psp.tile([128, M], F32, name="s", tag="s")
                        nc.tensor.matmul(
                            st[:, lo_i:M],
                            kj[j][:, (i % 4) * 128: (i % 4 + 1) * 128],
                            qt_t[j][:, lo_i:M],
                            start=True, stop=True, skip_group_check=True,
                        )
                        et = esbp.tile([128, M], BF16, name="e", tag=f"e{c}")
                        nc.scalar.activation(
                            et[:, lo_i:M], st[:, lo_i:M],
                            mybir.ActivationFunctionType.Exp,
                            bias=zbias[:],
                        )
                        if fhi[i] > lo_i:
                            nc.vector.tensor_mul(
                                et[:, lo_i: fhi[i]],
                                et[:, lo_i: fhi[i]],
                                mask_t[:, moff[i]: moff[i] + mwid[i]],
                            )
                        items.append((i, lo_i, et))
                    if items:
                        pend.append((j, ts, items))
                    if j >= 2 and pend:
                        flush_pv()

            # drain: flushes first so the recip chains pipeline, then norms
            tail_norms = []
            while pend:
                j, ts0 = flush_pv(norm_inline=False)
                if ts0 == NTS - 1:
                    tail_norms.append(j)
            for j in tail_norms:
                emit_norm(j, lo7, M)

        # remaining out-proj weights (space freed by the sweep pools)
        for dd in range(1, 4):
            wo_t.append(wop.tile([128, 2, D], BF16, name=f"wo{dd}", tag=f"wo{dd}"))
            nc.scalar.dma_start(wo_t[dd][:], wor[:, 2 * dd: 2 * dd + 2, :])


        # ---- phase C: y = O @ wo  (row-parallel partial) -----------
        with (
            nc.named_scope("phase_C"),
            tc.tile_pool(name="py", bufs=2, space="PSUM") as pyp,
            tc.tile_pool(name="ysb", bufs=3) as ysb,
        ):
            for mb in range(M // 128):
                for fp in range(D // 1024):
                    py = [
                        pyp.tile([128, 512], F32, name="py", tag=f"py{h}")
                        for h in range(2)
                    ]
                    for j in range(NHG):
                        for h in range(2):
                            fo = 2 * fp + h
                            nc.tensor.matmul(
                                py[h][:],
                                qt_t[j][:, mb * 128: (mb + 1) * 128],
                                wo_t[j // 2][:, j % 2, fo * 512: (fo + 1) * 512],
                                start=(j == 0),
                                stop=(j == NHG - 1),
                                skip_group_check=True,
                            )
                    for h in range(2):
                        ys = ysb.tile([128, 512], F32, name="ys", tag="ys")
                        nc.scalar.copy(ys[:], py[h][:])
                        eng = nc.sync if (2 * mb + fp + h) % 2 == 0 else nc.scalar
                        eng.dma_start(
                            y[
                                mb * 128: (mb + 1) * 128,
                                (2 * fp + h) * 512: (2 * fp + h + 1) * 512,
                            ],
                            ys[:],
                        )

    nc.compile()
    return nc


_cache = {}


def _get_program(flo, fhi):
    key = (tuple(flo), tuple(fhi))
    if key not in _cache:
        _cache[key] = build_program(list(flo), list(fhi))
    return _cache[key]


def _prep(inputs):
    x = np.asarray(inputs["x"], dtype=np.float32)
    qidx = np.asarray(inputs["query_idx"]).astype(np.int64)
    Wq = np.asarray(inputs["Wq"], dtype=np.float32)
    Wk = np.asarray(inputs["Wk"], dtype=np.float32)
    Wv = np.asarray(inputs["Wv"], dtype=np.float32)
    Wo = np.asarray(inputs["Wo"], dtype=np.float32)
    bq = np.asarray(inputs["bq"], dtype=np.float32)
    bk = np.asarray(inputs["bk"], dtype=np.float32)
    bv = np.asarray(inputs["bv"], dtype=np.float32)
    bo = np.asarray(inputs["bo"], dtype=np.float32)

    # Per-t-chunk skip bounds, union over batches.  flo[i] = first m that
    # attends into chunk i (everything below is fully masked there);
    # fhi[i] = one past the last m only partially covered by chunk i.
    # Computed positionally so they are correct even for unsorted
    # query_idx (just less effective at skipping).
    flo = [M] * NT
    fhi = [0] * NT
    for b in range(B):
        for i in range(NT):
            allowed = qidx[b] >= 128 * i          # chunk i not fully masked
            partial = qidx[b] < 128 * (i + 1)     # chunk i not fully allowed
            lo_b = int(np.argmax(allowed)) if allowed.any() else M
            hi_b = M - int(np.argmax(partial[::-1])) if partial.any() else 0
            flo[i] = min(flo[i], lo_b)
            fhi[i] = max(fhi[i], hi_b)

    # pack per-chunk mask windows [128, fhi-flo) into one [128, SW] tensor
    mwid = [max(fhi[i] - flo[i], 0) for i in range(NT)]
    moff = [0] * NT
    for i in range(1, NT):
        moff[i] = moff[i - 1] + mwid[i - 1]
    SW = max(moff[-1] + mwid[-1], 1)

    in_maps = []
    tgrid = np.arange(T)[:, None]
    for core in range(8):
        b, g = divmod(core, 2)
        sl = slice(g * DG, (g + 1) * DG)
        xb = x[b]
        mask = np.where(tgrid <= qidx[b][None, :], np.float32(1), np.float32(0))
        maskp = np.zeros((128, SW), dtype=np.float32)
        for i in range(NT):
            if mwid[i]:
                maskp[:, moff[i]: moff[i] + mwid[i]] = \
                    mask[128 * i: 128 * (i + 1), flo[i]: fhi[i]]
        in_maps.append(
            {
                "xT": np.ascontiguousarray(xb.T.astype(NPBF)),
                "xqT": np.ascontiguousarray(xb[qidx[b]].T.astype(NPBF)),
                "wk": np.ascontiguousarray(Wk[:, sl].astype(NPBF)),
                "wv": np.ascontiguousarray(Wv[:, sl].astype(NPBF)),
                "wq": np.ascontiguousarray(Wq[:, sl].astype(NPBF)),
                "wo": np.ascontiguousarray(Wo[sl, :].astype(NPBF)),
                "mask": np.ascontiguousarray(maskp.astype(NPBF)),
                "bks": np.ascontiguousarray(bk[sl].reshape(NHG, 128).T),
                "bqs": np.ascontiguousarray(
                    (bq[sl] / np.sqrt(HD)).reshape(NHG, 128).T.astype(np.float32)
                ),
            }
        )

    const = (bv.astype(np.float64) @ Wo.astype(np.float64) + bo).astype(np.float32)
    return flo, fhi, in_maps, const


def run(inputs, trace=False, trace_kwargs=None):
    _install_ntff_hook()
    flo, fhi, in_maps, const = _prep(inputs)
    nc = _get_program(flo, fhi)
    res = run_bass_kernel_spmd(
        nc, in_maps, list(range(8)), trace=trace, **(trace_kwargs or {})
    )
    out = np.zeros((B, M, D), dtype=np.float32)
    for b in range(B):
        out[b] = res.results[2 * b]["y"] + res.results[2 * b + 1]["y"] + const
    return out, res


def kernel(**inputs) -> np.ndarray:
    out, _ = run(inputs, trace=False)
    return out
